# revision 3
# baseline (speedup 1.0000x reference)
"""Trainium2 Bass kernel for nn_CascadingSinkCacheTriton.

The reference runs a sequential 4096-step scan per (n,h) lane maintaining a
cascading sink cache; the output is concat(cache_k, cache_v). Slot assignment
depends only on `score` and has an exact closed form (validated step-exactly
against the reference scan).

v3 design, driven by trace analysis (v1 122.9us, v2 102.5us):
  - DMA-engine payload is the binding resource (~24MB over 16 SDMA engines
    at ~24-27GB/s each). All reads are fp16 (rel err ~5e-4 << 2e-2 gate).
  - SWDGE descriptor runs are assigned per outermost-AP entry to a rotating
    engine cursor (+1 per entry, persists across calls): every det call is
    split into 2 calls of 8 outer entries so consecutive calls cover
    engines 0-7 / 8-15.
  - det cast-DMAs (DRAM fp16 -> DRAM f32, exact, line-rate) are issued
    AFTER the gathers: in v2 their ring descriptors stalled gather
    descriptor-generation by ~17us, shifting the whole tail chain.
  - One gather call per output column (1024 idxs) so each column's DVE
    convert + writeback chases its gather instead of waiting for a
    2-column batch.
  - Tiles are split per column: Tile tracks deps per-tensor, and monolithic
    pt/sel tiles serialized the DVE selects behind ALL pair loads.

Output image per lane: slot s = col*128 + p, 16 cols. Paths:
  - det cols {0..3, 14} + slots 1920..2044: gpsimd cast-DMA fp16->f32;
  - c1 pair cols {4..7}: fp16 pair rows (A|B contiguous), DVE select
    (B-A)*m + A with host 0/1 masks;
  - mixed cols {8..13}: fp16 SWDGE gathers, DVE tensor_copy fp16->f32,
    per-col contiguous writebacks;
  - slots 2045..2047: tiny tail gather.
"""

import numpy as np

# ---- problem constants (hardcoded per harness contract) ----
N, H, K, HID = 2, 32, 4096, 128
L = N * H                  # 64 lanes
T = 2048                   # cache slots per lane
ROW = 2 * HID              # 256 elems = 1 KB f32 / 512 B fp16 per row
WINDOW = 512
NCORES = 8
LPC = L // NCORES          # 8 lanes per core

NCALL = 128 * LPC          # idxs per 1-col gather call (1024)
NTL = 128                  # tail call (24 real + padding)
NIDX = (6 * NCALL + NTL) // 16
TAIL_SLOTS = [2045, 2046, 2047]


def _c1_a_rows() -> np.ndarray:
    """c1 A row for slot 512 + 128c + p: [4, 128] (B = A+1)."""
    a = np.empty((4, 128), np.int64)
    for c in range(4):
        sig = c * 128 + np.arange(128)
        a[c] = np.where(sig <= 507, 2568 + 2 * sig, 2560 + 2 * (sig - 508))
    return a


_A1 = _c1_a_rows()


# ------------------------------------------------------------------
# Host-side control flow: closed-form slot -> source-token-row map.
# ------------------------------------------------------------------
def _gather_indices(scores: np.ndarray) -> np.ndarray:
    """scores [L, K] f32 -> src [L, T] int64: 0-based token row per slot."""
    s = scores
    nl = s.shape[0]
    src = np.empty((nl, T), np.int64)

    def winner(x):
        return x + (s[:, x + 1] >= s[:, x])

    sig = np.arange(WINDOW)

    # cascade 0: deterministic, last 512 tokens
    src[:, 0:512] = (3584 + ((sig - 508) % 512))[None, :]

    # cascade 1: pairs (x, x+1), x = 3582 - 2*((507 - sig) % 512)
    src[:, 512:1024] = winner(3582 - 2 * ((507 - sig) % 512))

    # cascade 2
    c2 = np.empty((nl, WINDOW), np.int64)
    d2 = (sig - 509) % 512
    mp = d2 <= 254
    c2[:, mp] = winner(1026 + 2 * d2[mp])
    c2[:, 508] = winner(np.array([1024]))[:, 0]
    mq = (d2 >= 255) & (sig != 508)
    xq = 1536 + 4 * (d2[mq] - 255)
    wA = winner(xq)
    wB = winner(xq + 2)
    take_b = np.take_along_axis(s, wB, 1) >= np.take_along_axis(s, wA, 1)
    c2[:, mq] = np.where(take_b, wB, wA)
    src[:, 1024:1536] = c2

    # cascade 3
    c3 = np.empty((nl, WINDOW), np.int64)
    m = sig <= 251
    c3[:, m] = winner(519 + 2 * sig[m])
    c3[:, 252] = 1023
    m = (sig >= 253) & (sig <= 508)
    c3[:, m] = sig[m] + 4
    c3[:, 509:512] = winner(np.array([513, 515, 517]))
    src[:, 1536:2048] = c3

    return src


# ------------------------------------------------------------------
# Bass kernel (per core)
# ------------------------------------------------------------------
_NC_CACHE = {}


def _build_bass():
    if "nc" in _NC_CACHE:
        return _NC_CACHE["nc"]
    import concourse.bass as bass
    import concourse.bacc as bacc
    import concourse.tile as tile
    import concourse.mybir as mybir

    f32 = mybir.dt.float32
    f16 = mybir.dt.float16
    sub = mybir.AluOpType.subtract
    mult = mybir.AluOpType.mult
    add = mybir.AluOpType.add

    nc = bacc.Bacc("TRN2", target_bir_lowering=False, debug=False,
                   num_devices=NCORES)
    kv16 = nc.dram_tensor("kv16", [LPC * K, ROW], f16, kind="ExternalInput")
    idx = nc.dram_tensor("idx", [128, NIDX], mybir.dt.int16,
                         kind="ExternalInput")
    msk = nc.dram_tensor("msk", [128, 32], f16, kind="ExternalInput")
    out = nc.dram_tensor("out", [LPC, T, ROW], f32, kind="ExternalOutput")

    def out_ap(lane, slot, pattern):
        return bass.AP(out, (lane * T + slot) * ROW, pattern)

    def kv16_ap(row, pattern):
        return bass.AP(kv16, row * ROW, pattern)

    # fast writeback pattern: dram contiguous 128KB per (col, lane)
    def img_ap(col):
        return bass.AP(out, col * 128 * ROW,
                       [[ROW, 128], [T * ROW, LPC], [1, ROW]])

    # det cast-DMA: out slots [s0, s0+n) <- rows [r0, r0+n), all LPC lanes.
    # Emitted as 2 calls of 8 outer entries (n0 + rest rows): the SWDGE
    # engine cursor advances 1/entry, so the pair covers all 16 engines.
    def det_cast(s0, r0, n, n0):
        for d, m in ((0, n0), (n0, n - n0)):
            nc.gpsimd.dma_start(
                out=out_ap(0, s0 + d, [[T * ROW, LPC], [1, m * ROW]]),
                in_=kv16_ap(r0 + d, [[K * ROW, LPC], [1, m * ROW]]))

    with tile.TileContext(nc) as tc:
        with tc.tile_pool(name="pool", bufs=1) as pool:
            idx_sb = pool.tile([128, NIDX], mybir.dt.int16)
            msk_sb = pool.tile([128, 32], f16)
            # idx first: the gather chain is serial and critical
            nc.sync.dma_start(out=idx_sb[:], in_=idx[:])

            # ---- SWDGE gathers (fp16): one call per col {8..13} + tail ----
            gts = [pool.tile([128, LPC, ROW], f16, name=f"g{c}")
                   for c in range(6)]
            gtl = pool.tile([128, 1, ROW], f16)
            for i, (dst, n) in enumerate([(g, NCALL) for g in gts]
                                         + [(gtl, NTL)]):
                nc.gpsimd.dma_gather(
                    dst[:], kv16[:],
                    idx_sb[:, i * NCALL // 16:
                           i * NCALL // 16 + n // 16],
                    n, n, ROW, single_packet=False)

            # ---- det cols: fp16 -> f32 cast DMA, DRAM -> DRAM. After the
            # gathers: their ring descriptors would stall gather gen ----
            det_cast(0, 3588, 508, 254)     # c0 slots [0,508)
            det_cast(1792, 260, 128, 64)    # col 14
            det_cast(1920, 388, 125, 64)    # col 15a [1920,2045)
            nc.gpsimd.dma_start(            # c0 wrap [508,512), 8x4KB
                out=out_ap(0, 508, [[T * ROW, LPC], [1, 4 * ROW]]),
                in_=kv16_ap(3584, [[K * ROW, LPC], [1, 4 * ROW]]))

            # ---- c1 pair loads (fp16; A|B contiguous -> 1KB descs),
            # per-col tiles, split across both HWDGE queues ----
            pts = [pool.tile([128, LPC, 2 * ROW], f16, name=f"pt{c}")
                   for c in range(4)]
            nc.sync.dma_start(out=msk_sb[:], in_=msk[:])
            for c in range(4):
                q = nc.sync if c % 2 == 0 else nc.scalar
                q.dma_start(
                    out=pts[c][:],
                    in_=kv16_ap(2568 + 256 * c,
                                [[2 * ROW, 128], [K * ROW, LPC],
                                 [1, 2 * ROW]]))
            nc.sync.dma_start(      # col 7 p>=124 wrap: rows 2560..
                out=pts[3][124:128, :, :],
                in_=kv16_ap(2560, [[2 * ROW, 4], [K * ROW, LPC],
                                   [1, 2 * ROW]]))

            # ---- DVE select: out = (B - A) * m + A, then writeback ----
            sels = [pool.tile([128, LPC, ROW], f32, name=f"sel{c}")
                    for c in range(4)]
            dts = [pool.tile([128, LPC, ROW], f16, name=f"dt{c}")
                   for c in range(4)]
            for c in range(4):
                nc.vector.tensor_tensor(
                    out=dts[c][:],
                    in0=pts[c][:, :, ROW:2 * ROW],
                    in1=pts[c][:, :, 0:ROW], op=sub)
                for l in range(LPC):
                    nc.vector.scalar_tensor_tensor(
                        out=sels[c][:, l, :], in0=dts[c][:, l, :],
                        scalar=msk_sb[:, c * LPC + l:c * LPC + l + 1],
                        in1=pts[c][:, l, 0:ROW], op0=mult, op1=add)
                q = nc.sync if c % 2 == 0 else nc.scalar
                q.dma_start(out=img_ap(4 + c), in_=sels[c][:])

            # ---- gather converts (DVE fp16->f32) + per-col writebacks ----
            gfs = [pool.tile([128, LPC, ROW], f32, name=f"gf{c}")
                   for c in range(6)]
            gtf = pool.tile([128, 1, ROW], f32)
            for c in range(6):
                nc.vector.tensor_copy(out=gfs[c][:], in_=gts[c][:])
                q = nc.scalar if c % 2 == 0 else nc.sync
                q.dma_start(out=img_ap(8 + c), in_=gfs[c][:])
            nc.vector.tensor_copy(out=gtf[:], in_=gtl[:])
            for kk, slot in enumerate(TAIL_SLOTS):
                nc.scalar.dma_start(
                    out=out_ap(0, slot, [[T * ROW, LPC], [1, ROW]]),
                    in_=gtf[kk * LPC:(kk + 1) * LPC, 0, :])
    nc.compile()
    _NC_CACHE["nc"] = nc
    return nc


def _pack_idx(chunks) -> np.ndarray:
    """chunks: list of flat per-call gather sequences (row ids).
    -> [128, NIDX] int16: per-call 16-partition wrap, tiled x8."""
    parts = [c.astype(np.int16).reshape(-1, 16).T for c in chunks]
    return np.tile(np.concatenate(parts, axis=1), (8, 1))


def _make_in_maps(k, v, score):
    k = np.ascontiguousarray(k, np.float32).reshape(L, K, HID)
    v = np.ascontiguousarray(v, np.float32).reshape(L, K, HID)
    s = np.ascontiguousarray(score, np.float32).reshape(L, K)

    kv = np.concatenate([k, v], axis=-1)         # [L, K, 256] f32
    kv16 = kv.astype(np.float16)

    src = _gather_indices(s)                     # [L, T] token rows

    # sanity: det regions really are score-independent
    assert (src[:, 1792:1920] == np.arange(260, 388)).all()
    assert (src[:, 1920:2045] == np.arange(388, 513)).all()

    # select masks: m = src - A in {0,1}, [128 p, c*LPC + l]
    m1 = np.empty((L, 4, 128), np.int64)
    for c in range(4):
        m1[:, c] = src[:, (4 + c) * 128:(5 + c) * 128] - _A1[c]
    assert m1.min() >= 0 and m1.max() <= 1

    in_maps = []
    for core in range(NCORES):
        lanes = list(range(core * LPC, (core + 1) * LPC))
        # gather calls: one per col, i = l*128 + p -> slot col*128 + p
        chunks = []
        for col in (8, 9, 10, 11, 12, 13):
            seq = [src[lg, col * 128:(col + 1) * 128] + li * K
                   for li, lg in enumerate(lanes)]
            chunks.append(np.concatenate(seq))
        seq_t = np.zeros(NTL, np.int64)
        for kk, slot in enumerate(TAIL_SLOTS):
            for li, lg in enumerate(lanes):
                seq_t[kk * LPC + li] = src[lg, slot] + li * K
        chunks.append(seq_t)
        mco = np.empty((128, 32), np.float16)
        for c in range(4):
            for li, lg in enumerate(lanes):
                mco[:, c * LPC + li] = m1[lg, c]
        in_maps.append({
            "kv16": kv16[core * LPC:(core + 1) * LPC].reshape(LPC * K, ROW),
            "idx": _pack_idx(chunks),
            "msk": mco,
        })
    return in_maps


def kernel(k: np.ndarray, v: np.ndarray, score: np.ndarray) -> np.ndarray:
    from concourse.bass_utils import run_bass_kernel_spmd

    nc = _build_bass()
    in_maps = _make_in_maps(k, v, score)
    res = run_bass_kernel_spmd(nc, in_maps, list(range(NCORES)))
    return np.stack([r["out"] for r in res.results]).reshape(N, H, T, ROW)


def profile(k, v, score, tmpdir=None):
    """Run once with NTFF tracing; returns exec_time_ns (or None)."""
    from concourse.bass_utils import run_bass_kernel_spmd

    nc = _build_bass()
    in_maps = _make_in_maps(k, v, score)
    res = run_bass_kernel_spmd(nc, in_maps, list(range(NCORES)), trace=True,
                               tmpdir=tmpdir)
    return res.exec_time_ns


# revision 4
# speedup vs baseline: 1.2375x; 1.2375x over previous
"""Trainium2 Bass kernel for nn_CascadingSinkCacheTriton.

The reference runs a sequential 4096-step scan per (n,h) lane maintaining a
cascading sink cache; the output is concat(cache_k, cache_v). Slot assignment
depends only on `score` and has an exact closed form (validated step-exactly
against the reference scan).

v4 design, driven by trace analysis (v1 122.9us, v2 102.5us, v3 112.6us):
  - All reads are fp16 (rel err ~5e-4 << 2e-2 gate); f32 writes irreducible.
  - The serial cost chain on GPSIMD was the pacer: ~6us boot + ~15us
    DMAGatherAnt library reload + ~8ns/idx descriptor gen (50us for 6272
    idxs) + tail. Measured: gathers on DIFFERENT SWDGE queues generate
    descriptors in parallel (queue -> Q7 core-pair affinity), same-queue
    gathers serialize on gen+drain. So: num_swdge_queues=4, one 2-col
    gather per queue -> gen wall ~16us instead of ~50us.
  - det cast-DMAs (DRAM fp16 -> DRAM f32, exact, line-rate, resident
    ucode) are dispatched FIRST on queue 3: they generate before the
    library reload and drain during the otherwise-dead reload window
    without sitting ahead of gather descriptors in the same ring.
  - SWDGE descriptor runs are assigned per outermost-AP entry to a
    rotating per-queue engine cursor: every det region is emitted as 2
    calls of 8 outer entries so the pair covers all 16 SDMA engines.
  - Tiles are per-column: Tile tracks deps per-tensor, and monolithic
    tiles serialized the DVE selects behind ALL pair loads.

Output image per lane: slot s = col*128 + p, 16 cols. Paths:
  - det cols {0..3, 14} + slots 1920..2044: gpsimd cast-DMA fp16->f32;
  - c1 pair cols {4..7}: fp16 pair rows (A|B contiguous), DVE select
    (B-A)*m + A with host 0/1 masks;
  - mixed cols {8..13}: fp16 SWDGE gathers, DVE tensor_copy fp16->f32,
    per-col contiguous writebacks;
  - slots 2045..2047: tiny tail gather.
"""

import numpy as np

# ---- problem constants (hardcoded per harness contract) ----
N, H, K, HID = 2, 32, 4096, 128
L = N * H                  # 64 lanes
T = 2048                   # cache slots per lane
ROW = 2 * HID              # 256 elems = 1 KB f32 / 512 B fp16 per row
WINDOW = 512
NCORES = 8
LPC = L // NCORES          # 8 lanes per core

NCALL = 2 * 128 * LPC      # idxs per 2-col gather call (2048)
NTL = 128                  # tail call (24 real + padding)
NIDX = (3 * NCALL + NTL) // 16
TAIL_SLOTS = [2045, 2046, 2047]


def _c1_a_rows() -> np.ndarray:
    """c1 A row for slot 512 + 128c + p: [4, 128] (B = A+1)."""
    a = np.empty((4, 128), np.int64)
    for c in range(4):
        sig = c * 128 + np.arange(128)
        a[c] = np.where(sig <= 507, 2568 + 2 * sig, 2560 + 2 * (sig - 508))
    return a


_A1 = _c1_a_rows()


# ------------------------------------------------------------------
# Host-side control flow: closed-form slot -> source-token-row map.
# ------------------------------------------------------------------
def _gather_indices(scores: np.ndarray) -> np.ndarray:
    """scores [L, K] f32 -> src [L, T] int64: 0-based token row per slot."""
    s = scores
    nl = s.shape[0]
    src = np.empty((nl, T), np.int64)

    def winner(x):
        return x + (s[:, x + 1] >= s[:, x])

    sig = np.arange(WINDOW)

    # cascade 0: deterministic, last 512 tokens
    src[:, 0:512] = (3584 + ((sig - 508) % 512))[None, :]

    # cascade 1: pairs (x, x+1), x = 3582 - 2*((507 - sig) % 512)
    src[:, 512:1024] = winner(3582 - 2 * ((507 - sig) % 512))

    # cascade 2
    c2 = np.empty((nl, WINDOW), np.int64)
    d2 = (sig - 509) % 512
    mp = d2 <= 254
    c2[:, mp] = winner(1026 + 2 * d2[mp])
    c2[:, 508] = winner(np.array([1024]))[:, 0]
    mq = (d2 >= 255) & (sig != 508)
    xq = 1536 + 4 * (d2[mq] - 255)
    wA = winner(xq)
    wB = winner(xq + 2)
    take_b = np.take_along_axis(s, wB, 1) >= np.take_along_axis(s, wA, 1)
    c2[:, mq] = np.where(take_b, wB, wA)
    src[:, 1024:1536] = c2

    # cascade 3
    c3 = np.empty((nl, WINDOW), np.int64)
    m = sig <= 251
    c3[:, m] = winner(519 + 2 * sig[m])
    c3[:, 252] = 1023
    m = (sig >= 253) & (sig <= 508)
    c3[:, m] = sig[m] + 4
    c3[:, 509:512] = winner(np.array([513, 515, 517]))
    src[:, 1536:2048] = c3

    return src


# ------------------------------------------------------------------
# Bass kernel (per core)
# ------------------------------------------------------------------
_NC_CACHE = {}


def _build_bass():
    if "nc" in _NC_CACHE:
        return _NC_CACHE["nc"]
    import concourse.bass as bass
    import concourse.bacc as bacc
    import concourse.tile as tile
    import concourse.mybir as mybir

    f32 = mybir.dt.float32
    f16 = mybir.dt.float16
    sub = mybir.AluOpType.subtract
    mult = mybir.AluOpType.mult
    add = mybir.AluOpType.add

    nc = bacc.Bacc("TRN2", target_bir_lowering=False, debug=False,
                   num_devices=NCORES, num_swdge_queues=4)
    kv16 = nc.dram_tensor("kv16", [LPC * K, ROW], f16, kind="ExternalInput")
    idx = nc.dram_tensor("idx", [128, NIDX], mybir.dt.int16,
                         kind="ExternalInput")
    msk = nc.dram_tensor("msk", [128, 32], f16, kind="ExternalInput")
    out = nc.dram_tensor("out", [LPC, T, ROW], f32, kind="ExternalOutput")

    def out_ap(lane, slot, pattern):
        return bass.AP(out, (lane * T + slot) * ROW, pattern)

    def kv16_ap(row, pattern):
        return bass.AP(kv16, row * ROW, pattern)

    # fast writeback pattern: dram contiguous 128KB per (col, lane)
    def img_ap(col):
        return bass.AP(out, col * 128 * ROW,
                       [[ROW, 128], [T * ROW, LPC], [1, ROW]])

    # det cast-DMA: out slots [s0, s0+n) <- rows [r0, r0+n), all LPC lanes.
    # Emitted as 2 calls of 8 outer entries (n0 + rest rows): the SWDGE
    # engine cursor advances 1/entry, so the pair covers all 16 engines.
    def det_cast(s0, r0, n, n0):
        for d, m in ((0, n0), (n0, n - n0)):
            nc.gpsimd.dma_start(
                out=out_ap(0, s0 + d, [[T * ROW, LPC], [1, m * ROW]]),
                in_=kv16_ap(r0 + d, [[K * ROW, LPC], [1, m * ROW]]))

    with tile.TileContext(nc) as tc:
        with tc.tile_pool(name="pool", bufs=1) as pool:
            idx_sb = pool.tile([128, NIDX], mybir.dt.int16)
            msk_sb = pool.tile([128, 32], f16)
            # idx first: the gather chain is serial and critical
            nc.sync.dma_start(out=idx_sb[:], in_=idx[:])

            # ---- det cols: fp16 -> f32 cast DMA, DRAM -> DRAM, dispatched
            # first (resident ucode: generates before the gather library
            # reload, drains during it). queue 3: own ring, so the gather
            # descriptors on queues 0-2 never queue behind these ----
            # NOTE: bass dma_start has no queue_num; DIRECT2D d2d casts are
            # emitted on the default SWDGE queue 0 -- so give queue 0 the
            # LAST-dispatched gather (tail) to minimize in-ring conflict.
            det_cast(0, 3588, 508, 254)     # c0 slots [0,508)
            det_cast(1792, 260, 128, 64)    # col 14
            det_cast(1920, 388, 125, 64)    # col 15a [1920,2045)
            nc.gpsimd.dma_start(            # c0 wrap [508,512), 8x4KB
                out=out_ap(0, 508, [[T * ROW, LPC], [1, 4 * ROW]]),
                in_=kv16_ap(3584, [[K * ROW, LPC], [1, 4 * ROW]]))

            # ---- SWDGE gathers (fp16): cols {8,9} {10,11} {12,13} + tail,
            # one call per SWDGE queue -> descriptor gen runs in parallel ----
            g1 = pool.tile([128, 2 * LPC, ROW], f16)
            g2 = pool.tile([128, 2 * LPC, ROW], f16)
            g3 = pool.tile([128, 2 * LPC, ROW], f16)
            gt = pool.tile([128, 1, ROW], f16)
            for i, (dst, n, qn) in enumerate(((g1, NCALL, 1), (g2, NCALL, 2),
                                              (g3, NCALL, 3), (gt, NTL, 0))):
                nc.gpsimd.dma_gather(
                    dst[:], kv16[:],
                    idx_sb[:, i * NCALL // 16:
                           i * NCALL // 16 + n // 16],
                    n, n, ROW, single_packet=False, queue_num=qn)

            # ---- c1 pair loads (fp16; A|B contiguous -> 1KB descs),
            # per-col tiles, split across both HWDGE queues ----
            pts = [pool.tile([128, LPC, 2 * ROW], f16, name=f"pt{c}")
                   for c in range(4)]
            nc.sync.dma_start(out=msk_sb[:], in_=msk[:])
            for c in range(4):
                q = nc.sync if c % 2 == 0 else nc.scalar
                q.dma_start(
                    out=pts[c][:],
                    in_=kv16_ap(2568 + 256 * c,
                                [[2 * ROW, 128], [K * ROW, LPC],
                                 [1, 2 * ROW]]))
            nc.sync.dma_start(      # col 7 p>=124 wrap: rows 2560..
                out=pts[3][124:128, :, :],
                in_=kv16_ap(2560, [[2 * ROW, 4], [K * ROW, LPC],
                                   [1, 2 * ROW]]))

            # ---- DVE select: out = (B - A) * m + A, then writeback ----
            sels = [pool.tile([128, LPC, ROW], f32, name=f"sel{c}")
                    for c in range(4)]
            dts = [pool.tile([128, LPC, ROW], f16, name=f"dt{c}")
                   for c in range(4)]
            for c in range(4):
                nc.vector.tensor_tensor(
                    out=dts[c][:],
                    in0=pts[c][:, :, ROW:2 * ROW],
                    in1=pts[c][:, :, 0:ROW], op=sub)
                for l in range(LPC):
                    nc.vector.scalar_tensor_tensor(
                        out=sels[c][:, l, :], in0=dts[c][:, l, :],
                        scalar=msk_sb[:, c * LPC + l:c * LPC + l + 1],
                        in1=pts[c][:, l, 0:ROW], op0=mult, op1=add)
                q = nc.sync if c % 2 == 0 else nc.scalar
                q.dma_start(out=img_ap(4 + c), in_=sels[c][:])

            # ---- gather converts (DVE fp16->f32) + writebacks ----
            gfs = [pool.tile([128, 2 * LPC, ROW], f32, name=f"gf{i}")
                   for i in range(3)]
            gtf = pool.tile([128, 1, ROW], f32)
            for i, g in enumerate((g1, g2, g3)):
                for h in range(2):
                    nc.vector.tensor_copy(
                        out=gfs[i][:, h * LPC:(h + 1) * LPC, :],
                        in_=g[:, h * LPC:(h + 1) * LPC, :])
                    q = nc.scalar if h == 0 else nc.sync
                    q.dma_start(out=img_ap(8 + 2 * i + h),
                                in_=gfs[i][:, h * LPC:(h + 1) * LPC, :])
            nc.vector.tensor_copy(out=gtf[:], in_=gt[:])
            for kk, slot in enumerate(TAIL_SLOTS):
                nc.scalar.dma_start(
                    out=out_ap(0, slot, [[T * ROW, LPC], [1, ROW]]),
                    in_=gtf[kk * LPC:(kk + 1) * LPC, 0, :])
    nc.compile()
    _NC_CACHE["nc"] = nc
    return nc


def _pack_idx(chunks) -> np.ndarray:
    """chunks: list of flat per-call gather sequences (row ids).
    -> [128, NIDX] int16: per-call 16-partition wrap, tiled x8."""
    parts = [c.astype(np.int16).reshape(-1, 16).T for c in chunks]
    return np.tile(np.concatenate(parts, axis=1), (8, 1))


def _make_in_maps(k, v, score):
    k = np.ascontiguousarray(k, np.float32).reshape(L, K, HID)
    v = np.ascontiguousarray(v, np.float32).reshape(L, K, HID)
    s = np.ascontiguousarray(score, np.float32).reshape(L, K)

    kv = np.concatenate([k, v], axis=-1)         # [L, K, 256] f32
    kv16 = kv.astype(np.float16)

    src = _gather_indices(s)                     # [L, T] token rows

    # sanity: det regions really are score-independent
    assert (src[:, 1792:1920] == np.arange(260, 388)).all()
    assert (src[:, 1920:2045] == np.arange(388, 513)).all()

    # select masks: m = src - A in {0,1}, [128 p, c*LPC + l]
    m1 = np.empty((L, 4, 128), np.int64)
    for c in range(4):
        m1[:, c] = src[:, (4 + c) * 128:(5 + c) * 128] - _A1[c]
    assert m1.min() >= 0 and m1.max() <= 1

    in_maps = []
    for core in range(NCORES):
        lanes = list(range(core * LPC, (core + 1) * LPC))
        # gather calls: i = (c'*LPC + l)*128 + p -> slot (col0+c')*128 + p
        chunks = []
        for col0 in (8, 10, 12):
            seq = []
            for cp in range(2):
                for li, lg in enumerate(lanes):
                    seq.append(
                        src[lg, (col0 + cp) * 128:(col0 + cp + 1) * 128]
                        + li * K)
            chunks.append(np.concatenate(seq))
        seq_t = np.zeros(NTL, np.int64)
        for kk, slot in enumerate(TAIL_SLOTS):
            for li, lg in enumerate(lanes):
                seq_t[kk * LPC + li] = src[lg, slot] + li * K
        chunks.append(seq_t)
        mco = np.empty((128, 32), np.float16)
        for c in range(4):
            for li, lg in enumerate(lanes):
                mco[:, c * LPC + li] = m1[lg, c]
        in_maps.append({
            "kv16": kv16[core * LPC:(core + 1) * LPC].reshape(LPC * K, ROW),
            "idx": _pack_idx(chunks),
            "msk": mco,
        })
    return in_maps


def kernel(k: np.ndarray, v: np.ndarray, score: np.ndarray) -> np.ndarray:
    from concourse.bass_utils import run_bass_kernel_spmd

    nc = _build_bass()
    in_maps = _make_in_maps(k, v, score)
    res = run_bass_kernel_spmd(nc, in_maps, list(range(NCORES)))
    return np.stack([r["out"] for r in res.results]).reshape(N, H, T, ROW)


def profile(k, v, score, tmpdir=None):
    """Run once with NTFF tracing; returns exec_time_ns (or None)."""
    from concourse.bass_utils import run_bass_kernel_spmd

    nc = _build_bass()
    in_maps = _make_in_maps(k, v, score)
    res = run_bass_kernel_spmd(nc, in_maps, list(range(NCORES)), trace=True,
                               tmpdir=tmpdir)
    return res.exec_time_ns


# revision 5
# speedup vs baseline: 1.3615x; 1.1002x over previous
"""Trainium2 Bass kernel for nn_CascadingSinkCacheTriton.

The reference runs a sequential 4096-step scan per (n,h) lane maintaining a
cascading sink cache; the output is concat(cache_k, cache_v). Slot assignment
depends only on `score` and has an exact closed form (validated step-exactly
against the reference scan).

v5 design, driven by trace analysis (v1 122.9us, v2 102.5us, v4 91.0us):
  - All reads are fp16 (rel err ~5e-4 << 2e-2 gate); f32 writes irreducible.
    Total DMA payload ~24.3MB over 16 SDMA engines ~= 67us/engine busy
    floor; the remaining work is scheduling density.
  - GPSIMD serial chain: ~6us boot + ~10-15us DMAGatherAnt ucode reload
    (triggered at FIRST dma_gather dispatch) + descriptor gen. Gathers on
    different SWDGE queues gen in parallel (queue -> Q7 core-pair
    affinity); same-queue calls serialize on gen+drain. So: the tiny tail
    gather goes FIRST (own 2KB idx tensor, loaded in ~1us) purely to
    trigger the reload at ~2us; the six 1024-idx column gathers then
    round-robin queues 1/2/3.
  - det cast-DMAs (DRAM fp16 -> DRAM f32, exact, line-rate, resident
    ucode, 64KB descriptors) are dispatched AFTER the gathers on queue 0:
    their ~6.3MB drain fills the mid-kernel engine trough instead of
    starving the 1KB-descriptor HWDGE loads at the start (packet-
    granularity round-robin favors big packets).
  - SWDGE descriptor runs are assigned per outermost-AP entry to a
    rotating per-queue engine cursor: every det region is emitted as 2
    calls of 8 outer entries so the pair covers all 16 SDMA engines.
  - Tiles are per-column: Tile tracks deps per-tensor; coarse tiles
    serialized the DVE selects behind ALL pair loads.

Output image per lane: slot s = col*128 + p, 16 cols. Paths:
  - det cols {0..3, 14} + slots 1920..2044: gpsimd cast-DMA fp16->f32;
  - c1 pair cols {4..7}: fp16 pair rows (A|B contiguous), DVE select
    (B-A)*m + A with host 0/1 masks;
  - mixed cols {8..13}: fp16 SWDGE gathers, DVE tensor_copy fp16->f32,
    per-col contiguous writebacks;
  - slots 2045..2047: tiny tail gather.
"""

import numpy as np

# ---- problem constants (hardcoded per harness contract) ----
N, H, K, HID = 2, 32, 4096, 128
L = N * H                  # 64 lanes
T = 2048                   # cache slots per lane
ROW = 2 * HID              # 256 elems = 1 KB f32 / 512 B fp16 per row
WINDOW = 512
NCORES = 8
LPC = L // NCORES          # 8 lanes per core

NCALL = 128 * LPC          # idxs per 1-col gather call (1024)
NTL = 128                  # tail call (24 real + padding)
NIDX = 6 * NCALL // 16     # main idx tensor cols
TAIL_SLOTS = [2045, 2046, 2047]


def _c1_a_rows() -> np.ndarray:
    """c1 A row for slot 512 + 128c + p: [4, 128] (B = A+1)."""
    a = np.empty((4, 128), np.int64)
    for c in range(4):
        sig = c * 128 + np.arange(128)
        a[c] = np.where(sig <= 507, 2568 + 2 * sig, 2560 + 2 * (sig - 508))
    return a


_A1 = _c1_a_rows()


# ------------------------------------------------------------------
# Host-side control flow: closed-form slot -> source-token-row map.
# ------------------------------------------------------------------
def _gather_indices(scores: np.ndarray) -> np.ndarray:
    """scores [L, K] f32 -> src [L, T] int64: 0-based token row per slot."""
    s = scores
    nl = s.shape[0]
    src = np.empty((nl, T), np.int64)

    def winner(x):
        return x + (s[:, x + 1] >= s[:, x])

    sig = np.arange(WINDOW)

    # cascade 0: deterministic, last 512 tokens
    src[:, 0:512] = (3584 + ((sig - 508) % 512))[None, :]

    # cascade 1: pairs (x, x+1), x = 3582 - 2*((507 - sig) % 512)
    src[:, 512:1024] = winner(3582 - 2 * ((507 - sig) % 512))

    # cascade 2
    c2 = np.empty((nl, WINDOW), np.int64)
    d2 = (sig - 509) % 512
    mp = d2 <= 254
    c2[:, mp] = winner(1026 + 2 * d2[mp])
    c2[:, 508] = winner(np.array([1024]))[:, 0]
    mq = (d2 >= 255) & (sig != 508)
    xq = 1536 + 4 * (d2[mq] - 255)
    wA = winner(xq)
    wB = winner(xq + 2)
    take_b = np.take_along_axis(s, wB, 1) >= np.take_along_axis(s, wA, 1)
    c2[:, mq] = np.where(take_b, wB, wA)
    src[:, 1024:1536] = c2

    # cascade 3
    c3 = np.empty((nl, WINDOW), np.int64)
    m = sig <= 251
    c3[:, m] = winner(519 + 2 * sig[m])
    c3[:, 252] = 1023
    m = (sig >= 253) & (sig <= 508)
    c3[:, m] = sig[m] + 4
    c3[:, 509:512] = winner(np.array([513, 515, 517]))
    src[:, 1536:2048] = c3

    return src


# ------------------------------------------------------------------
# Bass kernel (per core)
# ------------------------------------------------------------------
_NC_CACHE = {}


def _build_bass():
    if "nc" in _NC_CACHE:
        return _NC_CACHE["nc"]
    import concourse.bass as bass
    import concourse.bacc as bacc
    import concourse.tile as tile
    import concourse.mybir as mybir

    f32 = mybir.dt.float32
    f16 = mybir.dt.float16
    sub = mybir.AluOpType.subtract
    mult = mybir.AluOpType.mult
    add = mybir.AluOpType.add

    nc = bacc.Bacc("TRN2", target_bir_lowering=False, debug=False,
                   num_devices=NCORES, num_swdge_queues=4)
    kv16 = nc.dram_tensor("kv16", [LPC * K, ROW], f16, kind="ExternalInput")
    idxt = nc.dram_tensor("idxt", [128, NTL // 16], mybir.dt.int16,
                          kind="ExternalInput")
    idx = nc.dram_tensor("idx", [128, NIDX], mybir.dt.int16,
                         kind="ExternalInput")
    msk = nc.dram_tensor("msk", [128, 32], f16, kind="ExternalInput")
    out = nc.dram_tensor("out", [LPC, T, ROW], f32, kind="ExternalOutput")

    def out_ap(lane, slot, pattern):
        return bass.AP(out, (lane * T + slot) * ROW, pattern)

    def kv16_ap(row, pattern):
        return bass.AP(kv16, row * ROW, pattern)

    # fast writeback pattern: dram contiguous 128KB per (col, lane)
    def img_ap(col):
        return bass.AP(out, col * 128 * ROW,
                       [[ROW, 128], [T * ROW, LPC], [1, ROW]])

    # det cast-DMA: out slots [s0, s0+n) <- rows [r0, r0+n), all LPC lanes.
    # Emitted as 2 calls of 8 outer entries (n0 + rest rows): the SWDGE
    # engine cursor advances 1/entry, so the pair covers all 16 engines.
    def det_cast(s0, r0, n, n0):
        for d, m in ((0, n0), (n0, n - n0)):
            nc.gpsimd.dma_start(
                out=out_ap(0, s0 + d, [[T * ROW, LPC], [1, m * ROW]]),
                in_=kv16_ap(r0 + d, [[K * ROW, LPC], [1, m * ROW]]))

    with tile.TileContext(nc) as tc:
        with tc.tile_pool(name="pool", bufs=1) as pool:
            idxt_sb = pool.tile([128, NTL // 16], mybir.dt.int16)
            idx_sb = pool.tile([128, NIDX], mybir.dt.int16)
            msk_sb = pool.tile([128, 32], f16)
            # tiny tail idx first: its gather dispatch triggers the ~10us
            # DMAGatherAnt ucode reload as early as possible
            nc.sync.dma_start(out=idxt_sb[:], in_=idxt[:])
            nc.sync.dma_start(out=idx_sb[:], in_=idx[:])
            gtl = pool.tile([128, 1, ROW], f16)
            nc.gpsimd.dma_gather(gtl[:], kv16[:], idxt_sb[:],
                                 NTL, NTL, ROW, single_packet=False,
                                 queue_num=0)

            # ---- SWDGE gathers (fp16): one call per col {8..13},
            # round-robin SWDGE queues 1-3 -> descriptor gen in parallel ----
            gts = [pool.tile([128, LPC, ROW], f16, name=f"g{c}")
                   for c in range(6)]
            for c in range(6):
                nc.gpsimd.dma_gather(
                    gts[c][:], kv16[:],
                    idx_sb[:, c * NCALL // 16:(c + 1) * NCALL // 16],
                    NCALL, NCALL, ROW, single_packet=False,
                    queue_num=1 + c % 3)

            # ---- det cols: fp16 -> f32 cast DMA, DRAM -> DRAM, queue 0.
            # After the gathers: their 64KB-descriptor drain fills the
            # mid-kernel engine trough instead of starving the early
            # 1KB-descriptor HWDGE loads ----
            det_cast(0, 3588, 508, 254)     # c0 slots [0,508)
            det_cast(1792, 260, 128, 64)    # col 14
            det_cast(1920, 388, 125, 64)    # col 15a [1920,2045)
            nc.gpsimd.dma_start(            # c0 wrap [508,512), 8x4KB
                out=out_ap(0, 508, [[T * ROW, LPC], [1, 4 * ROW]]),
                in_=kv16_ap(3584, [[K * ROW, LPC], [1, 4 * ROW]]))

            # ---- c1 pair loads (fp16; A|B contiguous -> 1KB descs),
            # per-col tiles, split across both HWDGE queues ----
            pts = [pool.tile([128, LPC, 2 * ROW], f16, name=f"pt{c}")
                   for c in range(4)]
            nc.sync.dma_start(out=msk_sb[:], in_=msk[:])
            for c in range(4):
                q = nc.sync if c % 2 == 0 else nc.scalar
                q.dma_start(
                    out=pts[c][:],
                    in_=kv16_ap(2568 + 256 * c,
                                [[2 * ROW, 128], [K * ROW, LPC],
                                 [1, 2 * ROW]]))
            nc.sync.dma_start(      # col 7 p>=124 wrap: rows 2560..
                out=pts[3][124:128, :, :],
                in_=kv16_ap(2560, [[2 * ROW, 4], [K * ROW, LPC],
                                   [1, 2 * ROW]]))

            # ---- DVE select: out = (B - A) * m + A, then writeback ----
            sels = [pool.tile([128, LPC, ROW], f32, name=f"sel{c}")
                    for c in range(4)]
            dts = [pool.tile([128, LPC, ROW], f16, name=f"dt{c}")
                   for c in range(4)]
            for c in range(4):
                nc.vector.tensor_tensor(
                    out=dts[c][:],
                    in0=pts[c][:, :, ROW:2 * ROW],
                    in1=pts[c][:, :, 0:ROW], op=sub)
                for l in range(LPC):
                    nc.vector.scalar_tensor_tensor(
                        out=sels[c][:, l, :], in0=dts[c][:, l, :],
                        scalar=msk_sb[:, c * LPC + l:c * LPC + l + 1],
                        in1=pts[c][:, l, 0:ROW], op0=mult, op1=add)
                q = nc.sync if c % 2 == 0 else nc.scalar
                q.dma_start(out=img_ap(4 + c), in_=sels[c][:])

            # ---- gather converts (DVE fp16->f32) + per-col writebacks ----
            gfs = [pool.tile([128, LPC, ROW], f32, name=f"gf{c}")
                   for c in range(6)]
            gtf = pool.tile([128, 1, ROW], f32)
            for c in range(6):
                nc.vector.tensor_copy(out=gfs[c][:], in_=gts[c][:])
                q = nc.scalar if c % 2 == 0 else nc.sync
                q.dma_start(out=img_ap(8 + c), in_=gfs[c][:])
            nc.vector.tensor_copy(out=gtf[:], in_=gtl[:])
            for kk, slot in enumerate(TAIL_SLOTS):
                nc.scalar.dma_start(
                    out=out_ap(0, slot, [[T * ROW, LPC], [1, ROW]]),
                    in_=gtf[kk * LPC:(kk + 1) * LPC, 0, :])
    nc.compile()
    _NC_CACHE["nc"] = nc
    return nc


def _pack_idx(chunks) -> np.ndarray:
    """chunks: list of flat per-call gather sequences (row ids).
    -> [128, sum/16] int16: per-call 16-partition wrap, tiled x8."""
    parts = [c.astype(np.int16).reshape(-1, 16).T for c in chunks]
    return np.tile(np.concatenate(parts, axis=1), (8, 1))


def _make_in_maps(k, v, score):
    k = np.ascontiguousarray(k, np.float32).reshape(L, K, HID)
    v = np.ascontiguousarray(v, np.float32).reshape(L, K, HID)
    s = np.ascontiguousarray(score, np.float32).reshape(L, K)

    kv = np.concatenate([k, v], axis=-1)         # [L, K, 256] f32
    kv16 = kv.astype(np.float16)

    src = _gather_indices(s)                     # [L, T] token rows

    # sanity: det regions really are score-independent
    assert (src[:, 1792:1920] == np.arange(260, 388)).all()
    assert (src[:, 1920:2045] == np.arange(388, 513)).all()

    # select masks: m = src - A in {0,1}, [128 p, c*LPC + l]
    m1 = np.empty((L, 4, 128), np.int64)
    for c in range(4):
        m1[:, c] = src[:, (4 + c) * 128:(5 + c) * 128] - _A1[c]
    assert m1.min() >= 0 and m1.max() <= 1

    in_maps = []
    for core in range(NCORES):
        lanes = list(range(core * LPC, (core + 1) * LPC))
        # gather calls: one per col, i = l*128 + p -> slot col*128 + p
        chunks = []
        for col in (8, 9, 10, 11, 12, 13):
            seq = [src[lg, col * 128:(col + 1) * 128] + li * K
                   for li, lg in enumerate(lanes)]
            chunks.append(np.concatenate(seq))
        seq_t = np.zeros(NTL, np.int64)
        for kk, slot in enumerate(TAIL_SLOTS):
            for li, lg in enumerate(lanes):
                seq_t[kk * LPC + li] = src[lg, slot] + li * K
        mco = np.empty((128, 32), np.float16)
        for c in range(4):
            for li, lg in enumerate(lanes):
                mco[:, c * LPC + li] = m1[lg, c]
        in_maps.append({
            "kv16": kv16[core * LPC:(core + 1) * LPC].reshape(LPC * K, ROW),
            "idx": _pack_idx(chunks),
            "idxt": _pack_idx([seq_t]),
            "msk": mco,
        })
    return in_maps


def kernel(k: np.ndarray, v: np.ndarray, score: np.ndarray) -> np.ndarray:
    from concourse.bass_utils import run_bass_kernel_spmd

    nc = _build_bass()
    in_maps = _make_in_maps(k, v, score)
    res = run_bass_kernel_spmd(nc, in_maps, list(range(NCORES)))
    return np.stack([r["out"] for r in res.results]).reshape(N, H, T, ROW)


def profile(k, v, score, tmpdir=None):
    """Run once with NTFF tracing; returns exec_time_ns (or None)."""
    from concourse.bass_utils import run_bass_kernel_spmd

    nc = _build_bass()
    in_maps = _make_in_maps(k, v, score)
    res = run_bass_kernel_spmd(nc, in_maps, list(range(NCORES)), trace=True,
                               tmpdir=tmpdir)
    return res.exec_time_ns
